# revision 2
# baseline (speedup 1.0000x reference)
"""Trainium2 Bass kernel v2 for the ViT block (B=64,N=197,C=768,H=12,P=20).

Data-parallel over batch (8 images/core). All big GEMMs run in fp8-e4m3 with
DoubleRow perf mode (K=256/matmul, 0.5 cyc/row). Tokens padded per-image to
W=200 (T=1600/core). Softmax denominators via a one-hot-column fp8 matmul on
e; normalization scale PE-broadcast; residuals via fused scalar_tensor_tensor;
LN stats via scaled-ones stationary matmuls (scale exactly representable in
fp8/bf16, compensated in activation scale args). Weights host-scaled by 32.
"""
import numpy as np
import concourse.bass as bass
import concourse.mybir as mybir
import concourse.tile as tile
from concourse import bacc, bass_utils
from contextlib import ExitStack

F32 = mybir.dt.float32
I32 = mybir.dt.int32
F32R = mybir.dt.float32r
BF16 = mybir.dt.bfloat16
FP8 = mybir.dt.float8e4
AF = mybir.ActivationFunctionType
DR = mybir.MatmulPerfMode.DoubleRow
OP = mybir.AluOpType

B, N, C, H, Dh, P, Dff = 64, 197, 768, 12, 64, 20, 3072
NCORES = 8
BL = B // NCORES
W = 200                  # padded per-image token width
TP = BL * W              # 1600 padded tokens per core
CH = 400                 # chunk (2 images)
NCH = TP // CH
CT = C // 128            # 6
KV2 = 256                # padded kv (197 tokens + 20 prompt + 39 zero)
EW = 208                 # e tile free width (200 + 8 pad, 16B aligned)
DOHW = 240               # denominator one-hot stationary width (>= 19*11+16)
EPS = 1e-6
S = 32.0                 # fp8 weight scale
C8 = 11.0 / 256.0        # e4m3-exact ones value for LN1 stats
A1 = 768.0 * C8          # = 33.0 exactly
SEXP = 1.0 / (8.0 * S * S)


def _bf16(v):
    import ml_dtypes
    return np.asarray(v, dtype=ml_dtypes.bfloat16)


def _c2():
    return float(np.float32(_bf16(np.float64(1.0 / 768.0))))


def build_nc(bl=BL):
    tp = bl * W
    nch = tp // CH
    A2 = 768.0 * _c2()

    nc = bacc.Bacc(trn_type="TRN2", target_bir_lowering=False)

    xb_d = nc.dram_tensor("xb", [C, tp], BF16, kind="ExternalInput")
    xr_d = nc.dram_tensor("xr", [C, tp], BF16, kind="ExternalInput")
    x8_d = nc.dram_tensor("x8", [C, tp], FP8, kind="ExternalInput")
    xq8_d = nc.dram_tensor("xq8", [C, tp], FP8, kind="ExternalInput")
    kp_d = nc.dram_tensor("kp", [CT, 128, bl, P], BF16, kind="ExternalInput")
    vp_d = nc.dram_tensor("vp", [bl, P, H, Dh], FP8, kind="ExternalInput")
    wqk_d = nc.dram_tensor("wqk", [C, 2 * C], FP8, kind="ExternalInput")
    wv_d = nc.dram_tensor("wv", [C, C], FP8, kind="ExternalInput")
    wpr_d = nc.dram_tensor("wpr", [C, C], FP8, kind="ExternalInput")
    wf1_d = nc.dram_tensor("wf1", [C, Dff], FP8, kind="ExternalInput")
    wf2_d = nc.dram_tensor("wf2", [Dff, C], FP8, kind="ExternalInput")
    wf1b_d = nc.dram_tensor("wf1b", [C, Dff], FP8, kind="ExternalInput")
    wf2b_d = nc.dram_tensor("wf2b", [Dff, C], FP8, kind="ExternalInput")
    doh_d = nc.dram_tensor("doh", [128, 2, DOHW], FP8, kind="ExternalInput")
    bh_d = nc.dram_tensor("bh", [12, 12 * 64], BF16, kind="ExternalInput")
    bqk_d = nc.dram_tensor("bqk", [128, 12], F32, kind="ExternalInput")
    bf1_d = nc.dram_tensor("bf1", [128, 24], F32, kind="ExternalInput")
    out_d = nc.dram_tensor("out_fm", [C, tp], F32, kind="ExternalOutput")

    def rearr(d):
        return d.rearrange("(kt p) m -> p kt m", p=128)

    with tile.TileContext(nc) as tc, ExitStack() as top:
        # ---------------- constants ----------------
        consts = top.enter_context(tc.tile_pool(name="consts", bufs=1))
        ones8 = consts.tile([128, 2, 16], FP8, name="ones8")
        nc.vector.memset(ones8[:], C8)
        ones2 = consts.tile([128, 1], BF16, name="ones2")
        nc.vector.memset(ones2[:], _c2())
        ones64b = consts.tile([1, 64], BF16, name="ones64b")
        nc.vector.memset(ones64b[:], 1.0)
        # denominator one-hot stationary (host-built): col 19h = S for real kv
        doh = consts.tile([128, 2, DOHW], FP8, name="doh")
        nc.sync.dma_start(doh[:], doh_d[:])
        bh = consts.tile([12, 12 * 64], BF16, name="bh")
        nc.sync.dma_start(bh[:], bh_d[:])
        eps_sb = consts.tile([1, 1], F32, name="eps_sb")
        nc.vector.memset(eps_sb[:], EPS)
        magic = consts.tile([1, CH], I32, name="magic")
        nc.vector.memset(magic[:], 0x5F3759DF)
        bqk_sb = consts.tile([128, 12], F32, name="bqk_sb")
        nc.sync.dma_start(bqk_sb[:], bqk_d[:])
        bf1_sb = consts.tile([128, 24], F32, name="bf1_sb")
        nc.sync.dma_start(bf1_sb[:], bf1_d[:])

        persist = top.enter_context(tc.tile_pool(name="persist", bufs=1))
        x2 = persist.tile([128, CT, tp], BF16, name="x2")
        rs2 = persist.tile([1, nch, CH], BF16, name="rs2")
        mrb2 = persist.tile([1, nch, CH], BF16, name="mrb2")

        mlpw = top.enter_context(tc.tile_pool(name="mlpw", bufs=1))
        wf1 = mlpw.tile([128, CT, Dff], FP8, name="wf1")
        wf2 = mlpw.tile([128, Dff // 128, C], FP8, name="wf2")
        bigP = top.enter_context(tc.tile_pool(name="bigP", bufs=3, space="PSUM"))

        def big():
            return bigP.tile([128, CH], F32, tag="big", name="big")

        attES = ExitStack()
        attLife = attES.enter_context(tc.tile_pool(name="attLife", bufs=1))
        q_sb = attLife.tile([128, CT, tp], BF16, name="q_sb")
        k_sb = attLife.tile([128, CT, bl, KV2], BF16, name="k_sb")
        v_sb = [attLife.tile([128, 2, H, Dh], FP8, name=f"v_sb{i}") for i in range(bl)]
        # zero pads once
        nc.gpsimd.memset(k_sb[:, :, :, P + N:KV2], 0.0)
        for im in range(bl):
            nc.gpsimd.memset(v_sb[im][64:128, 1, :, :], 0.0)
        for ct in range(CT):
            nc.sync.dma_start(k_sb[:, ct, :, N:N + P], kp_d[ct])

        # ---------------- phase A: LN1 + qkv + V ----------------
        phA = ExitStack()
        wqkp = phA.enter_context(tc.tile_pool(name="wqkp", bufs=1))
        wqk = wqkp.tile([128, CT, 2 * C], FP8, name="wqk")
        nc.sync.dma_start(wqk[:], rearr(wqk_d))
        wv = wqkp.tile([128, CT, C], FP8, name="wv")
        nc.sync.dma_start(wv[:], rearr(wv_d))

        pX = phA.enter_context(tc.tile_pool(name="pX", bufs=1, space="PSUM"))
        pV = phA.enter_context(tc.tile_pool(name="pV", bufs=2, space="PSUM"))

        with tc.tile_pool(name="lnA", bufs=2) as lnp, \
             tc.tile_pool(name="xs", bufs=2) as xsp:
            for j in range(nch):
                sl = slice(j * CH, (j + 1) * CH)
                x8t = xsp.tile([128, CT, CH], FP8, tag="x8", name="x8t")
                nc.sync.dma_start(x8t[:], rearr(x8_d)[:, :, sl])
                xq8t = xsp.tile([128, CT, CH], FP8, tag="xq8", name="xq8t")
                nc.sync.dma_start(xq8t[:], rearr(xq8_d)[:, :, sl])
                xbt = xsp.tile([128, CT, CH], BF16, tag="xb", name="xbt")
                nc.sync.dma_start(xbt[:], rearr(xb_d)[:, :, sl])

                s1 = pX.tile([1, CH], F32, tag="s1", name="s1")
                s2 = pX.tile([1, CH], F32, tag="s2", name="s2")
                for k in range(3):
                    nc.tensor.matmul(s1[:], ones8[:, :, 0:1], x8t[:, 2 * k:2 * k + 2, :],
                                     start=(k == 0), stop=(k == 2), perf_mode=DR)
                for k in range(3):
                    nc.tensor.matmul(s2[:], ones8[:, :, 0:1], xq8t[:, 2 * k:2 * k + 2, :],
                                     start=(k == 0), stop=(k == 2), perf_mode=DR)
                # musq' = (s1*A1^-.5)^2 = A1*mu^2; var' = s2 - musq' = A1*var
                musq = lnp.tile([1, CH], F32R, tag="musq", name="musq")
                nc.scalar.activation(out=musq[:], in_=s1[:], func=AF.Square,
                                     scale=float(A1 ** -0.5))
                var = lnp.tile([1, CH], F32R, tag="var", name="var")
                nc.vector.tensor_sub(var[:], s2[:], musq[:])
                sd = lnp.tile([1, CH], F32R, tag="sd", name="sd")
                nc.scalar.activation(out=sd[:], in_=var[:], func=AF.Sqrt,
                                     scale=float(1.0 / A1), bias=eps_sb[:])
                rs = lnp.tile([1, CH], BF16, tag="rs", name="rs")
                with nc.allow_low_precision(reason="rstd in bf16 is plenty"):
                    nc.vector.reciprocal(rs[:], sd[:])
                mrb = lnp.tile([1, CH], BF16, tag="mrb", name="mrb")
                nc.vector.scalar_tensor_tensor(out=mrb[:], in0=s1[:],
                                               scalar=float(1.0 / A1), in1=rs[:],
                                               op0=OP.mult, op1=OP.mult)
                rb_b = lnp.tile([128, 1, CH], BF16, tag="rb_b", name="rb_b")
                nc.gpsimd.partition_broadcast(rb_b[:, 0, :], rs[:])
                mrb_b = lnp.tile([128, 1, CH], BF16, tag="mrb_b", name="mrb_b")
                nc.gpsimd.partition_broadcast(mrb_b[:, 0, :], mrb[:])
                tmp = lnp.tile([128, CT, CH], BF16, tag="tmp", name="tmp")
                nc.vector.tensor_mul(tmp[:], xbt[:], rb_b[:].to_broadcast([128, CT, CH]))
                xh = lnp.tile([128, CT, CH], FP8, tag="xh", name="xh")
                nc.vector.tensor_sub(xh[:], tmp[:],
                                     mrb_b[:].to_broadcast([128, CT, CH]))

                # q/k GEMM for this chunk
                for mt in range(12):
                    ps = big()
                    for k in range(3):
                        nc.tensor.matmul(
                            ps[:], wqk[:, 2 * k:2 * k + 2, mt * 128:(mt + 1) * 128],
                            xh[:, 2 * k:2 * k + 2, :],
                            start=(k == 0), stop=(k == 2), perf_mode=DR)
                    if mt < 6:
                        nc.scalar.activation(out=q_sb[:, mt, sl], in_=ps[:],
                                             func=AF.Identity,
                                             bias=bqk_sb[:, mt:mt + 1])
                    else:
                        nc.scalar.activation(
                            out=k_sb[:, mt - 6, 2 * j:2 * j + 2, 0:N],
                            in_=ps[:].rearrange("p (v t) -> p v t", v=2)[:, :, 0:N],
                            func=AF.Identity, bias=bqk_sb[:, mt:mt + 1])
                # V GEMM (transposed out): tokens on partitions
                for v in range(2):
                    im = 2 * j + v
                    tc0 = v * W
                    for (toff, tsz, sub) in ((0, 128, 0), (128, 69, 1)):
                        for half in range(2):
                            pv = pV.tile([128, 384], F32, tag="pv", name="pv")
                            for k in range(3):
                                nc.tensor.matmul(
                                    pv[0:tsz, :],
                                    xh[:, 2 * k:2 * k + 2, tc0 + toff:tc0 + toff + tsz],
                                    wv[:, 2 * k:2 * k + 2, half * 384:(half + 1) * 384],
                                    start=(k == 0), stop=(k == 2), perf_mode=DR)
                            vdst = v_sb[im][0:tsz, sub, 6 * half:6 * half + 6, :]
                            vsrc = pv[0:tsz, :].rearrange("t (h d) -> t h d", d=Dh)
                            if (toff == 0) == (half == 0):
                                nc.vector.tensor_copy(vdst, vsrc)
                            else:
                                nc.scalar.activation(out=vdst, in_=vsrc,
                                                     func=AF.Identity)
        for im in range(bl):
            nc.sync.dma_start(v_sb[im][N - 128:N - 128 + P, 1, :, :], vp_d[im])

        phA.close()

        # ---------------- phase B: attention + proj + LN2 sums ----------------
        nc.sync.dma_start(wf1[:], rearr(wf1_d))
        nc.sync.dma_start(wf2[:], rearr(wf2_d))

        phB = ExitStack()
        wprp = phB.enter_context(tc.tile_pool(name="wprp", bufs=1))
        wpr = wprp.tile([128, CT, C], FP8, name="wpr")
        nc.sync.dma_start(wpr[:], rearr(wpr_d))
        o_fm = wprp.tile([128, CT, tp], FP8, name="o_fm")
        e_bufs = [[wprp.tile([128, 2, EW], FP8, name=f"e_{h}_{p}")
                   for p in range(2)] for h in range(H)]
        for h in range(H):
            for p in range(2):
                nc.gpsimd.memset(e_bufs[h][p][:, :, W:EW], 0.0)
        att = phB.enter_context(tc.tile_pool(name="att", bufs=2))
        xrp = phB.enter_context(tc.tile_pool(name="xrp", bufs=2))
        pAv = phB.enter_context(tc.tile_pool(name="pAv", bufs=2, space="PSUM"))
        pRb = phB.enter_context(tc.tile_pool(name="pRb", bufs=2, space="PSUM"))
        pDn = phB.enter_context(tc.tile_pool(name="pDn", bufs=1, space="PSUM"))

        r32s = {}
        xrts = {}

        def loop2_head(im, h):
            hp, hoff = h // 2, (h % 2) * 64
            av = pAv.tile([64, EW], F32, tag="av", name="av")
            nc.tensor.matmul(av[:], v_sb[im][:, :, h, :], e_bufs[h][im % 2][:],
                             start=True, stop=True, perf_mode=DR)
            rbp = pRb.tile([64, W], F32, tag="rb", name="rbp")
            nc.tensor.matmul(rbp[:], bh[:, 64 * h:64 * h + 64], r32s[im % 2][:],
                             start=True, stop=True)
            rbs = att.tile([64, W], BF16, tag=f"rbs{h % 2}", name="rbs")
            nc.scalar.activation(out=rbs[:], in_=rbp[:], func=AF.Identity)
            dst = o_fm[hoff:hoff + 64, hp, im * W:im * W + W]
            nc.vector.tensor_mul(dst, av[:, 0:W], rbs[:])

        def proj_chunk(j):
            sl = slice(j * CH, (j + 1) * CH)
            for mt in range(CT):
                pp = big()
                for k in range(3):
                    nc.tensor.matmul(pp[:], wpr[:, 2 * k:2 * k + 2, mt * 128:(mt + 1) * 128],
                                     o_fm[:, 2 * k:2 * k + 2, sl],
                                     start=(k == 0), stop=(k == 2), perf_mode=DR)
                nc.vector.scalar_tensor_tensor(
                    out=x2[:, mt, sl], in0=pp[:], scalar=float(1.0 / S),
                    in1=xrts[j % 2][:, mt, :], op0=OP.mult, op1=OP.add)
            x2q = att.tile([128, CT, CH], BF16, tag="x2q", name="x2q")
            nc.gpsimd.tensor_mul(x2q[:], x2[:, :, sl], x2[:, :, sl])
            s1 = big()
            s2 = big()
            for i in range(CT):
                nc.tensor.matmul(s1[0:1, :], ones2[:], x2[:, i, sl],
                                 start=(i == 0), stop=(i == CT - 1))
            for i in range(CT):
                nc.tensor.matmul(s2[0:1, :], ones2[:], x2q[:, i, :],
                                 start=(i == 0), stop=(i == CT - 1))
            musq = att.tile([1, CH], F32R, tag="musq", name="musq")
            nc.scalar.activation(out=musq[:], in_=s1[0:1, :], func=AF.Square)
            var = att.tile([1, CH], F32R, tag="var", name="var")
            nc.vector.tensor_sub(var[:], s2[0:1, :], musq[:])
            ve = att.tile([1, CH], F32R, tag="ve", name="ve")
            nc.vector.tensor_scalar_add(ve[:], var[:], EPS)
            sh = att.tile([1, CH], I32, tag="sh", name="sh")
            nc.vector.tensor_scalar(out=sh[:], in0=ve[:].bitcast(I32),
                                    scalar1=1, scalar2=None,
                                    op0=OP.logical_shift_right)
            y0 = att.tile([1, CH], I32, tag="y0", name="y0")
            nc.vector.scalar_tensor_tensor(out=y0[:], in0=magic[:], scalar=0,
                                           in1=sh[:], op0=OP.add,
                                           op1=OP.subtract)
            y0f = y0[:].bitcast(F32R)
            a = att.tile([1, CH], F32R, tag="a", name="a")
            nc.vector.tensor_mul(a[:], ve[:], y0f)
            b = att.tile([1, CH], F32R, tag="b", name="b")
            nc.vector.tensor_mul(b[:], a[:], y0f)
            w = att.tile([1, CH], F32R, tag="w", name="w")
            nc.vector.tensor_scalar(out=w[:], in0=b[:], scalar1=-0.5,
                                    scalar2=1.5, op0=OP.mult, op1=OP.add)
            with nc.allow_low_precision(reason="rstd in bf16"):
                nc.vector.tensor_mul(rs2[0:1, j, :], y0f, w[:])
            nc.vector.scalar_tensor_tensor(out=mrb2[0:1, j, :], in0=s1[0:1, :],
                                           scalar=1.0, in1=rs2[0:1, j, :],
                                           op0=OP.mult, op1=OP.mult)

        pend = None
        for im in range(bl):
            j = im // 2
            if im % 2 == 0:
                xrt = xrp.tile([128, CT, CH], BF16, tag="xr", name="xrt")
                nc.sync.dma_start(
                    xrt[:], rearr(xr_d)[:, :, j * CH:(j + 1) * CH])
                xrts[j % 2] = xrt
            t0 = im * W
            dn = pDn.tile([16, EW], F32, tag="dn", name="dn")
            for h in range(H):
                hp, hoff = h // 2, (h % 2) * 64
                q_ap = q_sb[hoff:hoff + 64, hp, t0:t0 + W]
                k_ap = k_sb[hoff:hoff + 64, hp, im, :]
                sb = big()
                sps = sb[:].rearrange("p (a t) -> p a t", a=2)
                nc.tensor.matmul(sps[:, 0, :], k_ap[:, 0:128], q_ap,
                                 start=True, stop=True)
                nc.tensor.matmul(sps[:, 1, :], k_ap[:, 128:KV2], q_ap,
                                 start=True, stop=True)
                e = e_bufs[h][im % 2]
                nc.scalar.activation(out=e[:, :, 0:W], in_=sps[:], func=AF.Exp,
                                     scale=SEXP)
                if h > 0:
                    nc.tensor.matmul(dn[:], doh[:, :, 19 * (h - 1):19 * (h - 1) + 16],
                                     e_bufs[h - 1][im % 2][:],
                                     start=(h == 1), stop=False, perf_mode=DR,
                                     skip_group_check=True)
                if pend is not None:
                    loop2_head(pend, h)
            nc.tensor.matmul(dn[:], doh[:, :, 19 * (H - 1):19 * (H - 1) + 16],
                             e_bufs[H - 1][im % 2][:],
                             start=False, stop=True, perf_mode=DR,
                             skip_group_check=True)
            r32 = att.tile([12, W], BF16, tag="r32", name="r32")
            with nc.allow_low_precision(reason="softmax recip in bf16"):
                nc.vector.reciprocal(r32[:], dn[0:12, 0:W])
            r32s[im % 2] = r32
            if pend is not None and pend % 2 == 1:
                proj_chunk(pend // 2)
            pend = im
        for h in range(H):
            loop2_head(pend, h)
        proj_chunk(nch - 1)
        phB.close()
        attES.close()

        # ---------------- phase C+D: LN2 math, apply, MLP ----------------
        phD = ExitStack()
        xh2p = phD.enter_context(tc.tile_pool(name="xh2p", bufs=1))
        xh2 = xh2p.tile([128, CT, tp], FP8, name="xh2")
        xh2l = xh2p.tile([128, CT, tp], FP8, name="xh2l")
        wf1b = xh2p.tile([128, CT, Dff], FP8, name="wf1b")
        nc.sync.dma_start(wf1b[:], rearr(wf1b_d))
        wf2b = xh2p.tile([128, Dff // 128, C], FP8, name="wf2b")
        nc.sync.dma_start(wf2b[:], rearr(wf2b_d))
        with tc.tile_pool(name="ln2p", bufs=2) as ln2p, \
             tc.tile_pool(name="gp", bufs=2) as gp, \
             tc.tile_pool(name="outp", bufs=2) as outp:
            for j in range(nch):
                sl = slice(j * CH, (j + 1) * CH)
                rb_b = ln2p.tile([128, 1, CH], BF16, tag="rb_b", name="rb_b")
                nc.gpsimd.partition_broadcast(rb_b[:, 0, :], rs2[0:1, j, :])
                mrb_b = ln2p.tile([128, 1, CH], BF16, tag="mrb_b", name="mrb_b")
                nc.gpsimd.partition_broadcast(mrb_b[:, 0, :], mrb2[0:1, j, :])
                tmp = ln2p.tile([128, CT, CH], BF16, tag="tmp", name="tmp")
                nc.vector.tensor_mul(tmp[:], x2[:, :, sl],
                                     rb_b[:].to_broadcast([128, CT, CH]))
                nc.vector.tensor_sub(xh2[:, :, sl], tmp[:],
                                     mrb_b[:].to_broadcast([128, CT, CH]))
                d1 = ln2p.tile([128, CT, CH], BF16, tag="d1", name="d1")
                nc.gpsimd.tensor_sub(d1[:], tmp[:],
                                     mrb_b[:].to_broadcast([128, CT, CH]))
                nc.gpsimd.tensor_sub(xh2l[:, :, sl], d1[:], xh2[:, :, sl])

                g = gp.tile([128, Dff // 128, CH], FP8, tag="g", name="g")
                for mt in range(Dff // 128):
                    fp = big()
                    terms = [(wf1, xh2), (wf1b, xh2), (wf1, xh2l)]
                    # order: hi*Whi, hi*Wlo, lo*Whi (lo computed on Pool, lands last)
                    for ti, (wt, xt) in enumerate(terms):
                        for k in range(3):
                            nc.tensor.matmul(
                                fp[:], wt[:, 2 * k:2 * k + 2, mt * 128:(mt + 1) * 128],
                                xt[:, 2 * k:2 * k + 2, sl],
                                start=(ti == 0 and k == 0),
                                stop=(ti == len(terms) - 1 and k == 2), perf_mode=DR)
                    nc.scalar.activation(out=g[:, mt, :], in_=fp[:], func=AF.Gelu,
                                         scale=float(1.0 / S),
                                         bias=bf1_sb[:, mt:mt + 1])
                for mt in range(CT):
                    op2 = big()
                    for ti, wt in enumerate([wf2, wf2b]):
                        for k in range(Dff // 256):
                            nc.tensor.matmul(
                                op2[:], wt[:, 2 * k:2 * k + 2, mt * 128:(mt + 1) * 128],
                                g[:, 2 * k:2 * k + 2, :],
                                start=(ti == 0 and k == 0),
                                stop=(ti == 1 and k == Dff // 256 - 1), perf_mode=DR)
                    ot = outp.tile([128, CH], F32, tag="ot", name="ot")
                    nc.vector.scalar_tensor_tensor(
                        out=ot[:], in0=op2[:], scalar=float(1.0 / S),
                        in1=x2[:, mt, sl], op0=OP.mult, op1=OP.add)
                    nc.sync.dma_start(out_d[mt * 128:(mt + 1) * 128, sl], ot[:])
        phD.close()

    nc.compile()
    return nc


_NC_CACHE = {}


def _get_nc(bl=BL):
    if bl not in _NC_CACHE:
        _NC_CACHE[bl] = build_nc(bl)
    return _NC_CACHE[bl]


def _host_prep(x, prompt, ln1_w, ln1_b, qkv_w, qkv_b, proj_w, proj_b,
               ln2_w, ln2_b, fc1_w, fc1_b, fc2_w, fc2_b, bl=BL, ncores=NCORES):
    import ml_dtypes
    FP8NP = ml_dtypes.float8_e4m3
    BF = ml_dtypes.bfloat16
    f8 = np.float64
    ln1_w, ln1_b = f8(ln1_w), f8(ln1_b)
    ln2_w, ln2_b = f8(ln2_w), f8(ln2_b)
    qkv_w8, fc1_w8, proj_w8 = f8(qkv_w), f8(fc1_w), f8(proj_w)

    wqk = np.ascontiguousarray((qkv_w8[:2 * C] * ln1_w).T * S).astype(np.float32)
    bqk = (S * (f8(qkv_b[:2 * C]) + qkv_w8[:2 * C] @ ln1_b)).astype(np.float32)
    bqk = bqk.reshape(12, 128).T.copy()
    wv = np.ascontiguousarray((qkv_w8[2 * C:] * ln1_w).T * S).astype(np.float32)
    bv = f8(qkv_b[2 * C:]) + qkv_w8[2 * C:] @ ln1_b
    wpr = np.ascontiguousarray(proj_w8.T * S).astype(np.float32)
    xshift = (f8(proj_b) + proj_w8 @ bv).astype(np.float32)   # rides the residual
    wf1 = np.ascontiguousarray((fc1_w8 * ln2_w).T * S).astype(np.float32)
    bf1 = (f8(fc1_b) + fc1_w8 @ ln2_b).astype(np.float32).reshape(24, 128).T.copy()
    wf2 = np.ascontiguousarray(f8(fc2_w).T * S).astype(np.float32)

    doh = np.zeros((128, 2, DOHW), np.float32)
    for h in range(H):
        doh[:, 0, 20 * h] = S
        doh[0:89, 1, 20 * h] = S
    bh = np.zeros((12, 12 * 64), np.float32)
    for h in range(H):
        bh[h, 64 * h:64 * h + 64] = 1.0
    wf1h = wf1.astype(FP8NP)
    wf2h = wf2.astype(FP8NP)
    wf1b = (wf1 - wf1h.astype(np.float32)).astype(FP8NP)
    wf2b = (wf2 - wf2h.astype(np.float32)).astype(FP8NP)
    shared = dict(
        wqk=wqk.astype(FP8NP), wv=wv.astype(FP8NP), wpr=wpr.astype(FP8NP),
        wf1=wf1h, wf2=wf2h, wf1b=wf1b, wf2b=wf2b, bqk=bqk, bf1=bf1,
        doh=doh.astype(FP8NP), bh=bh.astype(BF))

    x = np.float32(x).reshape(ncores, bl, N, C)
    prompt = np.float32(prompt).reshape(ncores, bl, P, 2, H, Dh)
    in_maps = []
    for cidx in range(ncores):
        xp = np.zeros((bl, W, C), np.float32)
        xp[:, :N, :] = x[cidx]
        x_fm = np.ascontiguousarray(xp.reshape(bl * W, C).T)   # [C, tp]
        xr = x_fm + xshift[:, None]
        in_maps.append(dict(
            xb=x_fm.astype(BF), xr=xr.astype(BF),
            x8=x_fm.astype(FP8NP), xq8=(x_fm * x_fm).astype(FP8NP),
            kp=np.ascontiguousarray(
                (S * prompt[cidx, :, :, 0]).reshape(bl, P, CT, 128)
                .transpose(2, 3, 0, 1)).astype(BF),
            vp=np.ascontiguousarray(S * prompt[cidx, :, :, 1]).astype(FP8NP),
            **shared))
    return in_maps


def run_sharded(inputs, bl=BL, ncores=NCORES, **spmd_kwargs):
    in_maps = _host_prep(**inputs, bl=bl, ncores=ncores)
    nc = _get_nc(bl)
    res = bass_utils.run_bass_kernel_spmd(nc, in_maps, core_ids=list(range(ncores)),
                                          **spmd_kwargs)
    fc2_b = np.float32(inputs["fc2_b"])
    outs = [r["out_fm"].T.reshape(bl, W, C)[:, :N, :] for r in res.results]
    out = np.concatenate(outs, axis=0).astype(np.float32) + fc2_b[None, None, :]
    return out, res


def kernel(**inputs):
    out, _ = run_sharded(inputs, bl=BL, ncores=NCORES)
    return out
